# revision 1
# baseline (speedup 1.0000x reference)
"""Expert-parallel MoE kernel for Trainium2 (8 NeuronCores).

Strategy (hardcoded for the nn_MoE problem: H=1024, E=8, top-k=2, I=1408,
shared-I=2816, T=2*2048=4096 tokens, f32 inputs):

- Expert parallel: core r owns routed expert r (dense compute over all T
  tokens, mathematically identical to the reference's dense einsum+combine).
- Shared expert is tensor-parallel: core r owns columns [r*352,(r+1)*352) of
  the shared intermediate dim.
- The gate (softmax top-2) is computed redundantly on every core in fp32 so
  routing decisions match the fp32 reference exactly; each core extracts the
  combine weight of its own expert (its gate matrix is permuted so its own
  expert sits in column 0).
- Each core produces partial = w_e(t)*expert_e(x)(t) + shared_partial(t) for
  all tokens, laid out as [H, T].  A ReduceScatter over the 8 cores sums the
  partials; core r ends up with rows [r*128,(r+1)*128) of y^T.  The host
  concatenates and transposes.
- All big matmuls run in bf16 with f32 PSUM accumulation; the gate runs in
  f32.  Work is split into 8 token chunks of 512 so the per-chunk
  ReduceScatter overlaps with compute of the following chunk.

Layouts put features on the partition axis and tokens on the free axis for
every matmul:
    up:   hg[i, t] = sum_h wg[h, i] * xT[h, t]     (lhsT=wg nat., rhs=xT nat.)
    down: eo[h, t] = sum_i wd[i, h] * act[i, t]    (lhsT=wd nat., rhs=act)
"""

import os
import sys

for _p in ("/opt/trn_rl_repo", "/root/.axon_site/_ro/trn_rl_repo"):
    if os.path.isdir(_p) and _p not in sys.path:
        sys.path.insert(0, _p)

import numpy as np

import concourse.bass as bass
import concourse.mybir as mybir
import concourse.tile as tile
from concourse import bacc
from concourse.bass_utils import run_bass_kernel_spmd

F32 = mybir.dt.float32
BF16 = mybir.dt.bfloat16
BF16_NP = mybir.dt.np(mybir.dt.bfloat16)
AX = mybir.AxisListType
ALU = mybir.AluOpType
ACTF = mybir.ActivationFunctionType

H = 1024          # hidden
E = 8             # experts = cores
I_R = 1408        # routed intermediate
SI = 352          # shared intermediate shard per core (2816 / 8)
N_CORES = 8
KC = H // 128     # 8 contraction chunks
IT_R = I_R // 128  # 11 routed intermediate tiles
SH_TILES = [(0, 0, 128), (1, 128, 128), (2, 256, 96)]  # shared i tiles
NEG_BIG = -1.0e30

LAST_RESULT = None  # BassKernelResults of the most recent run (for profiling)


def build_nc(T=4096, TC=512, trace_sim=False, silu_via_sigmoid=False):
    """Build the SPMD Bass program (identical on all 8 cores).

    silu_via_sigmoid: CoreSim has no Silu LUT; emulate it exactly as
    x*sigmoid(x) (an extra DVE multiply) for simulation runs only.
    """
    n_chunks = T // TC
    n_sub = TC // 128
    nc = bacc.Bacc("TRN2", target_bir_lowering=False, debug=False,
                   num_devices=N_CORES)

    xT = nc.dram_tensor("xT", [H, T], F32, kind="ExternalInput")
    # per-core gate slice: core r gets xT[:, r*T/8:(r+1)*T/8] (host-sliced)
    xg_d = nc.dram_tensor("xg", [H, T // N_CORES], F32, kind="ExternalInput")
    gwT = nc.dram_tensor("gwT", [H, E], F32, kind="ExternalInput")
    ident = nc.dram_tensor("ident", [128, 128], F32, kind="ExternalInput")
    wg = nc.dram_tensor("wg", [H, I_R], BF16, kind="ExternalInput")
    wu = nc.dram_tensor("wu", [H, I_R], BF16, kind="ExternalInput")
    wd = nc.dram_tensor("wd", [I_R, H], BF16, kind="ExternalInput")
    swg = nc.dram_tensor("swg", [H, SI], BF16, kind="ExternalInput")
    swu = nc.dram_tensor("swu", [H, SI], BF16, kind="ExternalInput")
    swd = nc.dram_tensor("swd", [SI, H], BF16, kind="ExternalInput")
    y = nc.dram_tensor("y", [128, T], F32, kind="ExternalOutput")

    rg = [list(range(N_CORES))]

    with tile.TileContext(nc, trace_sim=trace_sim) as tc:
        with (
            tc.tile_pool(name="const", bufs=1) as cpool,
            tc.tile_pool(name="xf", bufs=2) as xfpool,
            tc.tile_pool(name="xb", bufs=2) as xbpool,
            tc.tile_pool(name="gate", bufs=2) as gpool,
            tc.tile_pool(name="actr", bufs=2) as actrpool,
            tc.tile_pool(name="acts", bufs=2) as actspool,
            tc.tile_pool(name="tmp", bufs=3) as tpool,
            tc.tile_pool(name="eo", bufs=3) as eopool,
            tc.tile_pool(name="ps_small", bufs=3, space="PSUM") as ps_small,
            tc.tile_pool(name="ps_up", bufs=3, space="PSUM") as ps_up,
            tc.tile_pool(name="ps_o", bufs=2, space="PSUM") as ps_o,
            tc.tile_pool(name="dram", bufs=2, space="DRAM") as dpool,
        ):
            # ---- chunk-0 x + gate weights FIRST so PE starts early ----
            xf0 = xfpool.tile([128, KC, TC], F32, tag="xf")
            for k in range(KC):
                nc.sync.dma_start(xf0[:, k, :], xT[k * 128:(k + 1) * 128, 0:TC])
            gw_t = cpool.tile([128, KC, E], F32)
            for k in range(KC):
                nc.sync.dma_start(gw_t[:, k, :], gwT[k * 128:(k + 1) * 128, :])
            id_t = cpool.tile([128, 128], F32)
            nc.sync.dma_start(id_t[:, :], ident[:, :])
            ones = cpool.tile([1, 128], F32)
            nc.vector.memset(ones[:, :], 1.0)

            # ---- weights, split per contraction chunk so the first
            # up-proj matmuls only wait for their own slice ----
            wg_ks, wu_ks = [], []
            for k in range(KC):
                wgk = cpool.tile([128, I_R], BF16, tag=f"wg{k}")
                nc.sync.dma_start(wgk[:, :], wg[k * 128:(k + 1) * 128, :])
                wuk = cpool.tile([128, I_R], BF16, tag=f"wu{k}")
                nc.sync.dma_start(wuk[:, :], wu[k * 128:(k + 1) * 128, :])
                wg_ks.append(wgk)
                wu_ks.append(wuk)
            swg_ks, swu_ks = [], []
            for k in range(KC):
                sgk = cpool.tile([128, SI], BF16, tag=f"sg{k}")
                nc.sync.dma_start(sgk[:, :], swg[k * 128:(k + 1) * 128, :])
                suk = cpool.tile([128, SI], BF16, tag=f"su{k}")
                nc.sync.dma_start(suk[:, :], swu[k * 128:(k + 1) * 128, :])
                swg_ks.append(sgk)
                swu_ks.append(suk)
            wd_ts = []
            for it in range(IT_R):
                wdt = cpool.tile([128, H], BF16, tag=f"wd{it}")
                nc.sync.dma_start(wdt[:, :], wd[it * 128:(it + 1) * 128, :])
                wd_ts.append(wdt)
            swd_ts = []
            for it, m0, msz in SH_TILES:
                sdt = cpool.tile([128, H], BF16, tag=f"sd{it}")
                nc.sync.dma_start(sdt[:msz, :], swd[m0:m0 + msz, :])
                swd_ts.append(sdt)

            # ---- gate (sharded): each core computes the top-2 softmax
            # weights of ALL experts for ITS T/8-token slice, then one tiny
            # AllToAll redistributes so every core holds its OWN expert's
            # weight for ALL tokens, ordered by token (= chunk-major).
            GT = T // N_CORES
            a2a_in = dpool.tile([E, GT], F32, tag="a2ain")
            a2a_out = dpool.tile([E, GT], F32, tag="a2aout")
            n_gsub = (GT + 127) // 128
            wrow_all = gpool.tile([E, GT], F32, tag="wra")
            for j in range(n_gsub):
                g0 = j * 128
                gsz = min(128, GT - g0)
                xgt = gpool.tile([128, KC, 128], F32, tag="xgt")
                for k in range(KC):
                    nc.sync.dma_start(
                        xgt[:, k, :gsz], xg_d[k * 128:(k + 1) * 128,
                                              g0:g0 + gsz])
                pl = ps_small.tile([128, E], F32, tag="sm")
                for k in range(KC):
                    nc.tensor.matmul(
                        pl[:gsz, :], xgt[:, k, :gsz], gw_t[:, k, :],
                        start=(k == 0), stop=(k == KC - 1))
                lg = gpool.tile([128, E], F32, tag="lg")
                nc.vector.tensor_copy(lg[:gsz, :], pl[:gsz, :])
                m1 = gpool.tile([128, 1], F32, tag="m1")
                nc.vector.reduce_max(m1[:gsz, :], lg[:gsz, :], axis=AX.X)
                eq1 = gpool.tile([128, E], F32, tag="eq1")
                nc.vector.tensor_scalar(
                    eq1[:gsz, :], lg[:gsz, :], m1[:gsz, 0:1], None,
                    op0=ALU.is_equal)
                masked = gpool.tile([128, E], F32, tag="mk")
                nc.vector.scalar_tensor_tensor(
                    masked[:gsz, :], eq1[:gsz, :], NEG_BIG, lg[:gsz, :],
                    op0=ALU.mult, op1=ALU.add)
                m2l = gpool.tile([128, 1], F32, tag="m2l")
                nc.vector.reduce_max(m2l[:gsz, :], masked[:gsz, :], axis=AX.X)
                # w[:, e] = 1[l_e >= m2l] * sigmoid(2*l_e - m1 - m2l)
                arg = gpool.tile([128, E], F32, tag="arg")
                nc.vector.tensor_scalar_mul(arg[:gsz, :], lg[:gsz, :], 2.0)
                nc.vector.tensor_scalar(
                    arg[:gsz, :], arg[:gsz, :], m1[:gsz, 0:1], m2l[:gsz, 0:1],
                    op0=ALU.subtract, op1=ALU.subtract)
                sig = gpool.tile([128, E], F32, tag="sig")
                nc.scalar.activation(sig[:gsz, :], arg[:gsz, :], ACTF.Sigmoid)
                sel = gpool.tile([128, E], F32, tag="sel")
                nc.vector.tensor_scalar(
                    sel[:gsz, :], lg[:gsz, :], m2l[:gsz, 0:1], None,
                    op0=ALU.is_ge)
                wcol = gpool.tile([128, E], F32, tag="wc")
                nc.vector.tensor_mul(wcol[:gsz, :], sig[:gsz, :], sel[:gsz, :])
                ptr = ps_small.tile([E, 128], F32, tag="sm")
                nc.tensor.transpose(ptr[:, :gsz], wcol[:gsz, :],
                                    id_t[:gsz, :gsz])
                nc.vector.tensor_copy(wrow_all[:, g0:g0 + gsz], ptr[:, :gsz])
            nc.sync.dma_start(a2a_in[:, :], wrow_all[:, :])
            nc.gpsimd.collective_compute(
                "AllToAll", ALU.bypass, replica_groups=rg,
                ins=[a2a_in.opt()], outs=[a2a_out.opt()])
            # row-major element t of a2a_out is this expert's weight for
            # global token t

            for c in range(n_chunks):
                t0 = c * TC
                # ---- load x chunk (f32) and cast to bf16 ----
                if c == 0:
                    xf = xf0
                else:
                    xf = xfpool.tile([128, KC, TC], F32, tag="xf")
                    for k in range(KC):
                        nc.sync.dma_start(
                            xf[:, k, :], xT[k * 128:(k + 1) * 128, t0:t0 + TC])
                xb = xbpool.tile([128, KC, TC], BF16)
                nc.vector.tensor_copy(xb[:, :, :], xf[:, :, :])

                # ---- gate weight row for this chunk (from AllToAll) ----
                wrow = gpool.tile([1, TC], F32)
                if GT >= TC:
                    r0 = t0 // GT
                    o0 = t0 % GT
                    nc.sync.dma_start(
                        wrow[0:1, :], a2a_out[r0:r0 + 1, o0:o0 + TC])
                else:
                    for b in range(TC // GT):
                        r0 = (t0 + b * GT) // GT
                        nc.sync.dma_start(
                            wrow[0:1, b * GT:(b + 1) * GT],
                            a2a_out[r0:r0 + 1, :])
                # broadcast w over 128 partitions
                pw = ps_small.tile([128, TC], F32, tag="sm")
                nc.tensor.matmul(pw[:, :], ones[0:1, :], wrow[0:1, :],
                                 start=True, stop=True)
                wb = gpool.tile([128, TC], F32)
                nc.vector.tensor_copy(wb[:, :], pw[:, :])

                # ---- routed expert up-proj + swiglu (scaled by gate w) ----
                actr = actrpool.tile([128, IT_R, TC], BF16)
                for it in range(IT_R):
                    pg = ps_up.tile([128, TC], F32, tag="up")
                    for k in range(KC):
                        nc.tensor.matmul(
                            pg[:, :], wg_ks[k][:, it * 128:(it + 1) * 128],
                            xb[:, k, :], start=(k == 0), stop=(k == KC - 1))
                    pu = ps_up.tile([128, TC], F32, tag="up")
                    for k in range(KC):
                        nc.tensor.matmul(
                            pu[:, :], wu_ks[k][:, it * 128:(it + 1) * 128],
                            xb[:, k, :], start=(k == 0), stop=(k == KC - 1))
                    sg = tpool.tile([128, TC], F32, tag="sg")
                    if silu_via_sigmoid:
                        nc.scalar.activation(sg[:, :], pg[:, :], ACTF.Sigmoid)
                        nc.vector.tensor_mul(sg[:, :], sg[:, :], pg[:, :])
                    else:
                        nc.scalar.activation(sg[:, :], pg[:, :], ACTF.Silu)
                    tt = tpool.tile([128, TC], F32, tag="tt")
                    nc.vector.tensor_mul(tt[:, :], sg[:, :], pu[:, :])
                    nc.vector.tensor_mul(actr[:, it, :], tt[:, :], wb[:, :])

                # ---- shared expert shard up-proj + swiglu ----
                acts = actspool.tile([128, len(SH_TILES), TC], BF16)
                for it, m0, msz in SH_TILES:
                    pg = ps_up.tile([128, TC], F32, tag="up")
                    for k in range(KC):
                        nc.tensor.matmul(
                            pg[:msz, :], swg_ks[k][:, m0:m0 + msz],
                            xb[:, k, :], start=(k == 0), stop=(k == KC - 1))
                    pu = ps_up.tile([128, TC], F32, tag="up")
                    for k in range(KC):
                        nc.tensor.matmul(
                            pu[:msz, :], swu_ks[k][:, m0:m0 + msz],
                            xb[:, k, :], start=(k == 0), stop=(k == KC - 1))
                    sg = tpool.tile([128, TC], F32, tag="sg")
                    if silu_via_sigmoid:
                        nc.scalar.activation(sg[:msz, :], pg[:msz, :],
                                             ACTF.Sigmoid)
                        nc.vector.tensor_mul(sg[:msz, :], sg[:msz, :],
                                             pg[:msz, :])
                    else:
                        nc.scalar.activation(sg[:msz, :], pg[:msz, :],
                                             ACTF.Silu)
                    nc.vector.tensor_mul(acts[:msz, it, :], sg[:msz, :],
                                         pu[:msz, :])

                # ---- down-proj (routed + shared into one accumulator) ----
                ccin = dpool.tile([H, TC], F32, tag="ccin")
                for hc in range(KC):
                    h0 = hc * 128
                    po = ps_o.tile([128, TC], F32, tag="o")
                    for it in range(IT_R):
                        nc.tensor.matmul(
                            po[:, :], wd_ts[it][:, h0:h0 + 128],
                            actr[:, it, :], start=(it == 0), stop=False)
                    for it, m0, msz in SH_TILES:
                        nc.tensor.matmul(
                            po[:, :], swd_ts[it][:msz, h0:h0 + 128],
                            acts[:msz, it, :], start=False,
                            stop=(it == len(SH_TILES) - 1))
                    eo = eopool.tile([128, TC], F32)
                    nc.vector.tensor_copy(eo[:, :], po[:, :])
                    nc.sync.dma_start(ccin[h0:h0 + 128, :], eo[:, :])

                # ---- combine across cores: ReduceScatter this chunk ----
                ccout = dpool.tile([128, TC], F32, tag="ccout")
                nc.gpsimd.collective_compute(
                    "ReduceScatter", ALU.add, replica_groups=rg,
                    ins=[ccin.opt()], outs=[ccout.opt()])
                nc.sync.dma_start(y[:, t0:t0 + TC], ccout[:, :])

    nc.compile()
    return nc


def make_in_maps(x, gate_w, wg, wu, wd, swg, swu, swd, T=4096):
    xT = np.ascontiguousarray(
        x.reshape(-1, H).T).astype(np.float32)[:, :T]
    ident = np.eye(128, dtype=np.float32)
    in_maps = []
    GT = T // N_CORES
    gwT_g = np.ascontiguousarray(gate_w.T.astype(np.float32))
    for r in range(N_CORES):
        in_maps.append({
            "xT": xT,
            "xg": np.ascontiguousarray(xT[:, r * GT:(r + 1) * GT]),
            "gwT": gwT_g,
            "ident": ident,
            "wg": np.ascontiguousarray(wg[r]).astype(BF16_NP),
            "wu": np.ascontiguousarray(wu[r]).astype(BF16_NP),
            "wd": np.ascontiguousarray(wd[r]).astype(BF16_NP),
            "swg": np.ascontiguousarray(swg[:, r * SI:(r + 1) * SI]).astype(BF16_NP),
            "swu": np.ascontiguousarray(swu[:, r * SI:(r + 1) * SI]).astype(BF16_NP),
            "swd": np.ascontiguousarray(swd[r * SI:(r + 1) * SI, :]).astype(BF16_NP),
        })
    return in_maps


_NC_CACHE = {}


def kernel(x, gate_w, wg, wu, wd, swg, swu, swd):
    global LAST_RESULT
    x = np.asarray(x)
    B, S, _ = x.shape
    T = B * S
    if T not in _NC_CACHE:
        _NC_CACHE[T] = build_nc(T=T)
    nc = _NC_CACHE[T]
    in_maps = make_in_maps(
        np.asarray(x, np.float32), np.asarray(gate_w, np.float32),
        np.asarray(wg, np.float32), np.asarray(wu, np.float32),
        np.asarray(wd, np.float32), np.asarray(swg, np.float32),
        np.asarray(swu, np.float32), np.asarray(swd, np.float32), T=T)
    res = run_bass_kernel_spmd(nc, in_maps, core_ids=list(range(N_CORES)))
    LAST_RESULT = res
    yT = np.concatenate([res.results[r]["y"] for r in range(N_CORES)], axis=0)
    return np.ascontiguousarray(yT.T).reshape(B, S, H).astype(np.float32)



# revision 2
# speedup vs baseline: 2.4238x; 2.4238x over previous
"""Sparse expert-parallel MoE kernel for Trainium2 (8 NeuronCores).

Strategy (hardcoded for the nn_MoE problem: H=1024, E=8, top-k=2, I=1408,
shared-I=2816, T=2*2048=4096 tokens, f32 inputs):

- The gate (softmax top-2) is tiny (0.03% of FLOPs) and is evaluated on the
  host in float64; routing decisions match the f32 reference (min rank-2/3
  score gap for this problem's data is ~4e-5, far above f32 noise).
- Routed experts are EXPERT-PARALLEL with true top-2 sparsity: core r owns
  expert r and computes it only over the tokens routed to it (host-side
  gather -> padded capacity C, a multiple of 128).  This is 4x fewer FLOPs
  than dense all-expert compute.
- The shared expert is sharded 2x4: cores are split into 2 token-groups of
  4; within a group each core owns a 704-wide slice of the 2816 shared
  intermediate dim.  Partials are summed on the host.
- Combine: host scatter-adds  w_e(t) * expert_e(x_t)  (f32) plus the shared
  partial sums.  No on-device collectives.
- All matmuls run in bf16 with f32 PSUM accumulation (host pre-casts).

Layouts put features on the partition axis and tokens on the free axis:
    up:   hg[i, t] = sum_h wg[h, i] * xT[h, t]   (lhsT=wg nat., rhs=xT)
    down: eo[h, t] = sum_i wd[i, h] * act[i, t]  (lhsT=wd nat., rhs=act)
"""

import os
import sys

for _p in ("/opt/trn_rl_repo", "/root/.axon_site/_ro/trn_rl_repo"):
    if os.path.isdir(_p) and _p not in sys.path:
        sys.path.insert(0, _p)

import numpy as np

import concourse.bass as bass
import concourse.mybir as mybir
import concourse.tile as tile
from concourse import bacc
from concourse.bass_utils import run_bass_kernel_spmd

F32 = mybir.dt.float32
BF16 = mybir.dt.bfloat16
BF16_NP = mybir.dt.np(mybir.dt.bfloat16)
AX = mybir.AxisListType
ALU = mybir.AluOpType
ACTF = mybir.ActivationFunctionType

H = 1024           # hidden
E = 8              # experts = cores
TOP_K = 2
I_R = 1408         # routed intermediate
SI_TP = 704        # shared intermediate slice per core (2816 / 4)
TS = 2048          # shared-expert tokens per core (4096 / 2 groups)
N_CORES = 8
KC = H // 128      # 8 contraction chunks over hidden
IT_R = I_R // 128  # 11 routed intermediate tiles
# shared intermediate tiles for 704 = 5*128 + 64
SH_IT = [(0, 0, 128), (1, 128, 128), (2, 256, 128), (3, 384, 128),
         (4, 512, 128), (5, 640, 64)]
TC = 512           # token tile (PSUM bank = 512 f32)

LAST_RESULT = None  # BassKernelResults of the most recent run (for profiling)


def _chunks_of(n):
    out = [TC] * (n // TC)
    if n % TC:
        out.append(n % TC)
    return out


def build_nc(C, trace_sim=False, silu_via_sigmoid=False):
    """Build the SPMD Bass program (identical on all 8 cores).

    C: routed-token capacity per core (multiple of 128).
    silu_via_sigmoid: CoreSim has no Silu LUT; emulate as x*sigmoid(x).
    """
    nc = bacc.Bacc("TRN2", target_bir_lowering=False, debug=False,
                   num_devices=N_CORES)

    xr = nc.dram_tensor("xr", [H, C], BF16, kind="ExternalInput")
    xs = nc.dram_tensor("xs", [H, TS], BF16, kind="ExternalInput")
    wg = nc.dram_tensor("wg", [H, I_R], BF16, kind="ExternalInput")
    wu = nc.dram_tensor("wu", [H, I_R], BF16, kind="ExternalInput")
    wd = nc.dram_tensor("wd", [I_R, H], BF16, kind="ExternalInput")
    sg = nc.dram_tensor("sg", [H, SI_TP], BF16, kind="ExternalInput")
    su = nc.dram_tensor("su", [H, SI_TP], BF16, kind="ExternalInput")
    sd = nc.dram_tensor("sd", [SI_TP, H], BF16, kind="ExternalInput")
    yr = nc.dram_tensor("yr", [H, C], F32, kind="ExternalOutput")
    ys = nc.dram_tensor("ys", [H, TS], F32, kind="ExternalOutput")

    with tile.TileContext(nc, trace_sim=trace_sim) as tc:
        with (
            tc.tile_pool(name="const", bufs=1) as cpool,
            tc.tile_pool(name="xin", bufs=2) as xpool,
            tc.tile_pool(name="act", bufs=2) as actpool,
            tc.tile_pool(name="tmp", bufs=3) as tpool,
            tc.tile_pool(name="eo", bufs=3) as eopool,
            tc.tile_pool(name="ps_up", bufs=4, space="PSUM") as ps_up,
            tc.tile_pool(name="ps_o", bufs=2, space="PSUM") as ps_o,
        ):
            # ---- chunk-0 routed tokens + routed weights first so the PE
            # can start as early as possible ----
            xr0 = xpool.tile([128, KC, TC], BF16, tag="xr")
            c0 = min(TC, C)
            for k in range(KC):
                nc.sync.dma_start(xr0[:, k, :c0], xr[k * 128:(k + 1) * 128, 0:c0])
            wg_ks, wu_ks = [], []
            for k in range(KC):
                wgk = cpool.tile([128, I_R], BF16, tag=f"wg{k}")
                nc.sync.dma_start(wgk[:, :], wg[k * 128:(k + 1) * 128, :])
                wuk = cpool.tile([128, I_R], BF16, tag=f"wu{k}")
                nc.sync.dma_start(wuk[:, :], wu[k * 128:(k + 1) * 128, :])
                wg_ks.append(wgk)
                wu_ks.append(wuk)
            wd_ts = []
            for it in range(IT_R):
                wdt = cpool.tile([128, H], BF16, tag=f"wd{it}")
                nc.sync.dma_start(wdt[:, :], wd[it * 128:(it + 1) * 128, :])
                wd_ts.append(wdt)
            # shared-expert weights (needed ~100us in; DMA streams them
            # behind the routed weights)
            sg_ks, su_ks = [], []
            for k in range(KC):
                sgk = cpool.tile([128, SI_TP], BF16, tag=f"sg{k}")
                nc.sync.dma_start(sgk[:, :], sg[k * 128:(k + 1) * 128, :])
                suk = cpool.tile([128, SI_TP], BF16, tag=f"su{k}")
                nc.sync.dma_start(suk[:, :], su[k * 128:(k + 1) * 128, :])
                sg_ks.append(sgk)
                su_ks.append(suk)
            sd_ts = []
            for it, m0, msz in SH_IT:
                sdt = cpool.tile([128, H], BF16, tag=f"sd{it}")
                nc.sync.dma_start(sdt[:msz, :], sd[m0:m0 + msz, :])
                sd_ts.append(sdt)

            def swiglu_chunk(x_t, n, gate_ks, up_ks, it_list, act_t):
                """act[i, :n] = silu(gate) * up over this token chunk."""
                for it, m0, msz in it_list:
                    pg = ps_up.tile([128, TC], F32, tag="up")
                    for k in range(KC):
                        nc.tensor.matmul(
                            pg[:msz, :n], gate_ks[k][:, m0:m0 + msz],
                            x_t[:, k, :n], start=(k == 0), stop=(k == KC - 1))
                    pu = ps_up.tile([128, TC], F32, tag="up")
                    for k in range(KC):
                        nc.tensor.matmul(
                            pu[:msz, :n], up_ks[k][:, m0:m0 + msz],
                            x_t[:, k, :n], start=(k == 0), stop=(k == KC - 1))
                    sa = tpool.tile([128, TC], F32, tag="sa")
                    if silu_via_sigmoid:
                        nc.scalar.activation(sa[:msz, :n], pg[:msz, :n],
                                             ACTF.Sigmoid)
                        nc.vector.tensor_mul(sa[:msz, :n], sa[:msz, :n],
                                             pg[:msz, :n])
                    else:
                        nc.scalar.activation(sa[:msz, :n], pg[:msz, :n],
                                             ACTF.Silu)
                    nc.vector.tensor_mul(act_t[:msz, it, :n], sa[:msz, :n],
                                         pu[:msz, :n])

            def down_chunk(act_t, n, down_ts, it_list, out_d, t0):
                for hc in range(KC):
                    h0 = hc * 128
                    po = ps_o.tile([128, TC], F32, tag="o")
                    last = len(it_list) - 1
                    for j, (it, m0, msz) in enumerate(it_list):
                        nc.tensor.matmul(
                            po[:, :n], down_ts[it][:msz, h0:h0 + 128],
                            act_t[:msz, it, :n], start=(j == 0),
                            stop=(j == last))
                    eo = eopool.tile([128, TC], F32)
                    nc.vector.tensor_copy(eo[:, :n], po[:, :n])
                    nc.sync.dma_start(out_d[h0:h0 + 128, t0:t0 + n], eo[:, :n])

            r_it = [(it, it * 128, 128) for it in range(IT_R)]

            # ---- routed expert over gathered tokens ----
            t0 = 0
            for ci, n in enumerate(_chunks_of(C)):
                if ci == 0:
                    x_t = xr0
                else:
                    x_t = xpool.tile([128, KC, TC], BF16, tag="xr")
                    for k in range(KC):
                        nc.sync.dma_start(
                            x_t[:, k, :n], xr[k * 128:(k + 1) * 128, t0:t0 + n])
                act_t = actpool.tile([128, IT_R, TC], BF16, tag="act")
                swiglu_chunk(x_t, n, wg_ks, wu_ks, r_it, act_t)
                down_chunk(act_t, n, wd_ts, r_it, yr, t0)
                t0 += n

            # ---- shared expert slice over this core's token group ----
            t0 = 0
            for n in _chunks_of(TS):
                x_t = xpool.tile([128, KC, TC], BF16, tag="xr")
                for k in range(KC):
                    nc.sync.dma_start(
                        x_t[:, k, :n], xs[k * 128:(k + 1) * 128, t0:t0 + n])
                act_t = actpool.tile([128, IT_R, TC], BF16, tag="act")
                swiglu_chunk(x_t, n, sg_ks, su_ks, SH_IT, act_t)
                down_chunk(act_t, n, sd_ts, SH_IT, ys, t0)
                t0 += n

    nc.compile()
    return nc


def _route_host(xf, gate_w):
    """Replicate the reference MoEGate exactly (float64 for determinism)."""
    logits = xf.astype(np.float64) @ gate_w.astype(np.float64).T
    m = logits.max(axis=-1, keepdims=True)
    ex = np.exp(logits - m)
    sc = ex / ex.sum(axis=-1, keepdims=True)
    topi = np.argsort(-sc, axis=-1, kind="stable")[:, :TOP_K]   # ties: low idx
    topw = np.take_along_axis(sc, topi, axis=-1)
    topw = topw / (topw.sum(axis=-1, keepdims=True) + 1e-20)    # SCALE = 1.0
    return topi, topw.astype(np.float64)


_NC_CACHE = {}


def kernel(x, gate_w, wg, wu, wd, swg, swu, swd):
    global LAST_RESULT
    x = np.asarray(x, np.float32)
    B, S, _ = x.shape
    T = B * S
    xf = x.reshape(T, H)

    # ---- host gate + dispatch ----
    topi, topw = _route_host(xf, np.asarray(gate_w, np.float32))
    e_ids = topi.ravel()
    t_ids = np.repeat(np.arange(T), TOP_K)
    w_all = topw.ravel()
    order = np.argsort(e_ids, kind="stable")
    e_sorted = e_ids[order]
    t_sorted = t_ids[order]
    w_sorted = w_all[order]
    counts = np.bincount(e_sorted, minlength=E)
    starts = np.concatenate([[0], np.cumsum(counts)])
    C = max(128, int(-(-counts.max() // 128)) * 128)

    if C not in _NC_CACHE:
        _NC_CACHE[C] = build_nc(C)
    nc = _NC_CACHE[C]

    xfT_bf = np.ascontiguousarray(xf.T).astype(BF16_NP)   # [H, T]
    wg = np.asarray(wg, np.float32)
    wu = np.asarray(wu, np.float32)
    wd = np.asarray(wd, np.float32)
    swg = np.asarray(swg, np.float32)
    swu = np.asarray(swu, np.float32)
    swd = np.asarray(swd, np.float32)

    in_maps = []
    idx_r = []
    w_r = []
    for r in range(N_CORES):
        lo, hi = starts[r], starts[r + 1]
        idx = t_sorted[lo:hi]
        idx_r.append(idx)
        w_r.append(w_sorted[lo:hi])
        xr = np.zeros((H, C), dtype=BF16_NP)
        xr[:, :len(idx)] = xfT_bf[:, idx]
        g, q = divmod(r, 4)
        in_maps.append({
            "xr": xr,
            "xs": np.ascontiguousarray(xfT_bf[:, g * TS:(g + 1) * TS]),
            "wg": np.ascontiguousarray(wg[r]).astype(BF16_NP),
            "wu": np.ascontiguousarray(wu[r]).astype(BF16_NP),
            "wd": np.ascontiguousarray(wd[r]).astype(BF16_NP),
            "sg": np.ascontiguousarray(
                swg[:, q * SI_TP:(q + 1) * SI_TP]).astype(BF16_NP),
            "su": np.ascontiguousarray(
                swu[:, q * SI_TP:(q + 1) * SI_TP]).astype(BF16_NP),
            "sd": np.ascontiguousarray(
                swd[q * SI_TP:(q + 1) * SI_TP, :]).astype(BF16_NP),
        })

    res = run_bass_kernel_spmd(nc, in_maps, core_ids=list(range(N_CORES)))
    LAST_RESULT = res

    # ---- host combine: shared partial sums + weighted routed scatter ----
    yT = np.zeros((H, T), np.float32)
    for g in range(2):
        acc = yT[:, g * TS:(g + 1) * TS]
        for q in range(4):
            acc += res.results[g * 4 + q]["ys"]
    for r in range(N_CORES):
        n = len(idx_r[r])
        if n:
            yT[:, idx_r[r]] += (res.results[r]["yr"][:, :n]
                                * w_r[r][None, :].astype(np.float32))
    return np.ascontiguousarray(yT.T).reshape(B, S, H).astype(np.float32)
